# revision 7
# baseline (speedup 1.0000x reference)
"""Trainium2 Bass kernel v2 for nn_BasicLSTM (B=64, T=512, D=512, U=1024).

Data-parallel over batch across 8 NeuronCores (8 sequences/core, recurrence
local per core).  Differences vs v1:

- The x-projection xz = x@Wx (plus bias) is no longer folded into every
  step's PSUM accumulation (5 of 13 K-chunks).  It is computed by a
  pipelined full-width GEMM (M=128 tiles of 16 timesteps x 8 batch) running
  two tiles ahead of the recurrence, staged in SBUF, and re-striped into a
  9-partition inject buffer by an SBUF->SBUF DMA.  Each step then injects
  xz_t + bias into the PSUM bank with a single K=9 matmul (identity over the
  8 batch rows + a ones row scaled by the bias).  Per-step PE work drops
  from 26 to ~20 N=512-column slots.
- Steps are ordered so bank A's z finishes ~1.9us into the step, giving the
  sigmoid -> cell-update -> tanh -> h -> transpose chain maximal overlap
  with bank B / GEMM matmuls.
- The elementwise chain is split across DVE and Pool (gpsimd) so no single
  engine exceeds ~2.7us/step: DVE does g2 = 2*sig-1 and f*c and h = o*tanh(c);
  Pool does i*g2, the c accumulation, and the GEMM PSUM->SBUF copies.

Weight columns host-permuted to [i|f|o|g] per 512-unit bank, g columns
pre-scaled by 2 (tanh(x) = 2*sigmoid(2x)-1).  Matmul operands bf16 (fp32
PSUM accumulation); the cell state c stays fp32.
"""

import numpy as np
import ml_dtypes

B, T, D, U = 64, 512, 512, 1024
NCORES = 8
BL = B // NCORES          # 8 sequences per core
NK_X = D // 128           # 4 x K-chunks
NK_H = U // 128           # 8 h K-chunks
NT = 512                  # N-tile width (one PSUM bank)
GATE_OFF = (0, 32, 64, 96)  # PSUM partition offset per col-group (i,f,o,g)
TPT = 16                  # timesteps per GEMM tile (16*BL = 128 = M)


def _build_nc(t_steps=T, lite=False):
    import concourse.bass as bass
    import concourse.mybir as mybir

    f32, bf16 = mybir.dt.float32, mybir.dt.bfloat16
    AF = mybir.ActivationFunctionType
    ALU = mybir.AluOpType

    ntiles = (t_steps + TPT - 1) // TPT
    t_pad = ntiles * TPT
    n_pro_tiles = min(2, ntiles)   # GEMM tiles computed before the step loop

    nc = bass.Bass(num_devices=NCORES)
    if lite:
        wq = nc.declare_dram_parameter("wq", [1, 64], bf16, isOutput=False)
        xq = nc.declare_dram_parameter("xq", [1, 64], bf16, isOutput=False)
    else:
        wq = nc.declare_dram_parameter("wq", [1537, 4096], bf16, isOutput=False)
        xq = nc.declare_dram_parameter("xq", [NK_X, 128, t_pad, BL], bf16, isOutput=False)
    cst_d = nc.declare_dram_parameter("cst", [72, 528], bf16, isOutput=False)
    cstf_d = nc.declare_dram_parameter("cstf", [72, 8], f32, isOutput=False)
    out_d = nc.declare_dram_parameter("out", [BL, U], f32, isOutput=True)

    from contextlib import ExitStack
    ctx = ExitStack()
    sb = lambda shape, dt, name: ctx.enter_context(nc.sbuf_tensor(name, shape, dt))
    ps = lambda shape, dt, name: ctx.enter_context(nc.psum_tensor(name, shape, dt))
    sem = lambda name: ctx.enter_context(nc.semaphore(name))

    with ctx:
        w_sb = sb([128, 12 * 4096], bf16, "w_sb")
        x_sb = sb([128, NK_X * t_pad * BL], bf16, "x_sb")
        cst = sb([72, 528], bf16, "cst_sb")    # inj lhsT @[0:9,0:8]; I8 @[64:72,8:16]; zeros @[:,16:528]
        cstf = sb([72, 8], f32, "cstf_sb")     # I8 at rows 32:40 and 64:72
        st_sb = [sb([128, 4096], bf16, f"st_sb{i}") for i in range(3)]
        ib_sb = [sb([9, 4096], bf16, f"ib_sb{i}") for i in range(2)]
        s_sb = [sb([128, NT], bf16, f"s_sb{b}") for b in range(2)]
        g2_sb = sb([BL, 2 * NT], bf16, "g2_sb")
        c1_sb = sb([32 + BL, 2 * NT], f32, "c1_sb")
        t1_sb = sb([32 + BL, 2 * NT], f32, "t1_sb")
        c_sb = sb([32 + BL, 2 * NT], f32, "c_sb")
        tc_sb = sb([64 + BL, 2 * NT], bf16, "tc_sb")
        tcT_sb = sb([128, 2 * 32], bf16, "tcT_sb")
        hT_sb = sb([128, 2 * NK_H * BL], bf16, "hT_sb")  # dbl-buffered h.T
        hlast_sb = sb([BL, U], f32, "hlast_sb")

        zbuf = [ps([128, NT], f32, f"zbuf{i}") for i in range(4)]
        gm = [ps([128, NT], f32, f"gm{i}") for i in range(2)]
        tr_o = ps([128, 2 * 32], bf16, "tr_o")
        tr_c = ps([128, 2 * 32], f32, "tr_c")

        dma_sem = sem("dma_sem")       # init loads
        injdma_sem = sem("injdma_sem")  # per-step inject restripe DMA
        zmm_sem = sem("zmm_sem")       # z accumulation complete (2/step)
        gmm_sem = sem("gmm_sem")       # GEMM n-chunk complete (1/n-chunk)
        pcopy_sem = sem("pcopy_sem")   # GEMM PSUM->SBUF copy (1/n-chunk)
        sig_sem = sem("sig_sem")       # 2/step
        g2_sem = sem("g2_sem")         # 2/step
        c1_sem = sem("c1_sem")         # 2/step
        csum_sem = sem("csum_sem")     # 2/step
        tanh_sem = sem("tanh_sem")     # 2/step (tanh on transposed c)
        trO_sem = sem("trO_sem")       # o transposes done (2/step)
        trC_sem = sem("trC_sem")       # c transposes done (2/step)
        hT_sem = sem("hT_sem")         # hT chunks ready (2/step)
        hlast_sem = sem("hlast_sem")   # final h written (2)

        n_init = 2 if lite else (12 + NK_X + 4)

        # GEMM MM helper data: global n-chunk index N -> PSUM bank N%2
        def gemm_mm(tensor, g, k):
            # k in 0..31 within tile g: n-chunk k//4, K-chunk k%4
            n, kc = k // 4, k % 4
            Ng = g * 8 + n
            z = gm[Ng % 2]
            if kc == 0:
                tensor.wait_ge(dma_sem, 16 * n_init)
                if Ng >= 2:
                    tensor.wait_ge(pcopy_sem, Ng - 1)
            lhsT = x_sb[:, kc * (t_pad * BL) + g * 128:kc * (t_pad * BL) + (g + 1) * 128]
            ins = tensor.matmul(
                z[:, :], lhsT, w_sb[:, kc * 4096 + n * NT:kc * 4096 + (n + 1) * NT],
                start=(kc == 0), stop=(kc == 3),
                skip_group_check=True,
            )
            if kc == 3:
                ins.then_inc(gmm_sem, 1)

        with nc.Block() as block:

            @block.sync
            def _(sync):
                if not lite:
                    for kc in range(12):
                        sync.dma_start(
                            out=w_sb[:, kc * 4096:(kc + 1) * 4096],
                            in_=wq[kc * 128:(kc + 1) * 128, :],
                        ).then_inc(dma_sem, 16)
                    for kc in range(NK_X):
                        sync.dma_start(
                            out=x_sb[:, kc * (t_pad * BL):(kc + 1) * (t_pad * BL)],
                            in_=xq[kc],
                        ).then_inc(dma_sem, 16)
                    sync.dma_start(out=ib_sb[0][8:9, :], in_=wq[1536:1537, :]).then_inc(dma_sem, 16)
                    sync.dma_start(out=ib_sb[1][8:9, :], in_=wq[1536:1537, :]).then_inc(dma_sem, 16)
                sync.dma_start(out=cst[:, :], in_=cst_d[:, :]).then_inc(dma_sem, 16)
                sync.dma_start(out=cstf[:, :], in_=cstf_d[:, :]).then_inc(dma_sem, 16)
                # per-step inject restripe: st tile -> 8-partition inject buf
                for t in range(t_steps):
                    sync.wait_ge(pcopy_sem, 8 * (t // TPT + 1))
                    if t >= 1:
                        # serialize inject DMA completions (sem-race safety)
                        sync.wait_ge(injdma_sem, 16 * t)
                    if t >= 2:
                        # ib_sb[t%2] free only once BOTH banks' z of t-2 done
                        sync.wait_ge(zmm_sem, 2 * (t - 2) + 2)
                    tl = t % TPT
                    sync.dma_start(
                        out=ib_sb[t % 2][0:BL, :],
                        in_=st_sb[(t // TPT) % 3][BL * tl:BL * (tl + 1), :],
                    ).then_inc(injdma_sem, 16)
                # final store
                sync.wait_ge(hlast_sem, 2)
                sync.dma_start(out=out_d[:, :], in_=hlast_sb[:, :]).then_inc(dma_sem, 16)

            @block.tensor
            def _(tensor):
                # prologue: GEMM tiles 0..n_pro_tiles-1
                for g in range(n_pro_tiles):
                    for k in range(32):
                        gemm_mm(tensor, g, k)
                tensor.wait_ge(dma_sem, 16 * n_init)

                def inject_bank(tensor, t, bk, z):
                        if t < 2:
                            # one-time full-bank clear so later reads of the
                            # non-gate partitions see initialized memory
                            tensor.matmul(
                                z[:, :], cst[0:1, 16:144], cst[0:1, 16:528],
                                start=True, stop=False,
                                skip_group_check=True,
                            )
                        # inject xz_t + bias (K=9: I8 rows + ones*bias row)
                        tensor.wait_ge(injdma_sem, 16 * (t + 1))
                        if t >= 2:
                            tensor.wait_ge(sig_sem, 2 * (t - 2) + 2)
                        for cg in range(4):
                            ncol = (4 * bk + cg) * NT
                            ins = tensor.matmul(
                                z[GATE_OFF[cg]:GATE_OFF[cg] + BL, :],
                                cst[0:9, 0:8],
                                ib_sb[t % 2][0:9, ncol:ncol + NT],
                                start=True, stop=(t == 0),
                                tile_position=(0, GATE_OFF[cg]),
                                skip_group_check=True,
                            )
                            if t == 0 and cg == 3:
                                ins.then_inc(zmm_sem, 1)
                        if t == 0:
                            return
                        # 8 h K-chunks; first 4 from hTmulA(t-1), rest B
                        for j in range(NK_H):
                            if j == 0:
                                tensor.wait_ge(hT_sem, 2 * (t - 1) + 1)
                            elif j == 4:
                                tensor.wait_ge(hT_sem, 2 * (t - 1) + 2)
                            lhsT = hT_sb[:, ((t - 1) % 2) * (NK_H * BL) + j * BL:
                                         ((t - 1) % 2) * (NK_H * BL) + (j + 1) * BL]
                            kc = NK_X + j
                            for cg in range(4):
                                ncol = (4 * bk + cg) * NT
                                last = (j == NK_H - 1)
                                ins = tensor.matmul(
                                    z[GATE_OFF[cg]:GATE_OFF[cg] + BL, :],
                                    lhsT,
                                    w_sb[:, kc * 4096 + ncol:kc * 4096 + ncol + NT],
                                    start=False, stop=last,
                                    tile_position=(0, GATE_OFF[cg]),
                                    skip_group_check=True,
                                )
                                if last and cg == 3:
                                    ins.then_inc(zmm_sem, 1)

                def tr_o_bank(tensor, t, bk):
                    tensor.wait_ge(sig_sem, 2 * t + bk + 1)
                    if t >= 1:
                        tensor.wait_ge(hT_sem, 2 * (t - 1) + bk + 1)
                    for j in range(4):
                        ins = tensor.matmul(
                            tr_o[:, bk * 32 + j * BL:bk * 32 + (j + 1) * BL],
                            s_sb[bk][64:64 + BL, j * 128:(j + 1) * 128],
                            cst[64:72, 8:16],
                            start=True, stop=True,
                            is_transpose=True,
                            skip_group_check=True,
                        )
                        if j == 3:
                            ins.then_inc(trO_sem, 1)

                def tr_c_bank(tensor, t, bk):
                    tensor.wait_ge(csum_sem, 2 * t + bk + 1)
                    if t >= 1:
                        tensor.wait_ge(tanh_sem, 2 * (t - 1) + bk + 1)
                    for j in range(4):
                        ins = tensor.matmul(
                            tr_c[:, bk * 32 + j * BL:bk * 32 + (j + 1) * BL],
                            c_sb[32:32 + BL,
                                 bk * NT + j * 128:bk * NT + (j + 1) * 128],
                            cstf[32:40, 0:8],
                            start=True, stop=True,
                            is_transpose=True,
                            skip_group_check=True,
                        )
                        if j == 3:
                            ins.then_inc(trC_sem, 1)

                for t in range(t_steps):
                    zA = zbuf[(t % 2) * 2]
                    zB = zbuf[(t % 2) * 2 + 1]
                    # bank A z first (chain head), transposes interleaved so
                    # the PE never idles long enough to re-throttle HAM and
                    # the chain tail starts as early as possible
                    inject_bank(tensor, t, 0, zA)
                    if t < t_steps - 1:
                        tr_o_bank(tensor, t, 0)
                    inject_bank(tensor, t, 1, zB)
                    if t < t_steps - 1:
                        tr_c_bank(tensor, t, 0)
                    g = t // TPT + 2
                    if g < ntiles:
                        for d_ in range(2):
                            gemm_mm(tensor, g, 2 * (t % TPT) + d_)
                    if t < t_steps - 1:
                        tr_o_bank(tensor, t, 1)
                        tr_c_bank(tensor, t, 1)

            @block.scalar
            def _(scalar):
                # prologue GEMM staging copies (PSUM -> SBUF; GPSIMD can't
                # touch PSUM, ACT has the most headroom)
                for g in range(n_pro_tiles):
                    for n in range(8):
                        Ng = g * 8 + n
                        scalar.wait_ge(gmm_sem, Ng + 1)
                        nc.scalar.copy(
                            st_sb[g % 3][:, n * NT:(n + 1) * NT], gm[Ng % 2][:, :],
                        ).then_inc(pcopy_sem, 1)
                for t in range(t_steps):
                    zA = zbuf[(t % 2) * 2]
                    zB = zbuf[(t % 2) * 2 + 1]
                    for bk, z in ((0, zA), (1, zB)):
                        scalar.wait_ge(zmm_sem, 2 * t + bk + 1)
                        if t >= 1:
                            # s_sb readers of step t-1 must be done: DVE
                            # (csum last), Pool c1, PE o-transposes
                            scalar.wait_ge(csum_sem, 2 * (t - 1) + 2)
                            scalar.wait_ge(c1_sem, 2 * (t - 1) + 2)
                            scalar.wait_ge(trO_sem, 2 * (t - 1) + bk + 1)
                        nc.scalar.activation(
                            s_sb[bk][:, :], z[:, :], AF.Sigmoid,
                        ).then_inc(sig_sem, 1)
                    # staging copies after the sigmoids so they never delay
                    # the chain head (tanh waits on csum anyway)
                    g = t // TPT + 2
                    if g < ntiles:
                        s_off = t % TPT
                        ns = [n for n in range(8)
                              if (2 * n + 2 == s_off) or (n == 7 and s_off == 15)]
                        for n in ns:
                            Ng = g * 8 + n
                            scalar.wait_ge(gmm_sem, Ng + 1)
                            if g >= 3:
                                scalar.wait_ge(injdma_sem, 16 * TPT * (g - 2))
                            nc.scalar.copy(
                                st_sb[g % 3][:, n * NT:(n + 1) * NT],
                                gm[Ng % 2][:, :],
                            ).then_inc(pcopy_sem, 1)
                    for bk in range(2):
                        if t < t_steps - 1:
                            # tanh on the transposed c pieces [128, 32]
                            scalar.wait_ge(trC_sem, 2 * t + bk + 1)
                            if t >= 1:
                                scalar.wait_ge(hT_sem, 2 * (t - 1) + bk + 1)
                            nc.scalar.activation(
                                tcT_sb[:, bk * 32:(bk + 1) * 32],
                                tr_c[:, bk * 32:(bk + 1) * 32],
                                AF.Tanh,
                            ).then_inc(tanh_sem, 1)
                        else:
                            scalar.wait_ge(csum_sem, 2 * t + bk + 1)
                            nc.scalar.activation(
                                tc_sb[64:64 + BL, bk * NT:(bk + 1) * NT],
                                c_sb[32:32 + BL, bk * NT:(bk + 1) * NT],
                                AF.Tanh,
                            ).then_inc(tanh_sem, 1)

            @block.vector
            def _(vector):
                for t in range(t_steps):
                    for bk in range(2):
                        vector.wait_ge(sig_sem, 2 * t + bk + 1)
                        # g2 = 2*sigmoid(2*zg) - 1 = tanh(zg); shift 96 -> 0
                        nc.vector.tensor_scalar(
                            g2_sb[:, bk * NT:(bk + 1) * NT],
                            s_sb[bk][96:96 + BL, :], 2.0, -1.0,
                            ALU.mult, ALU.add,
                        )
                        vector.drain()
                        nc.vector.tensor_mul(
                            t1_sb[32:32 + BL, bk * NT:(bk + 1) * NT],
                            s_sb[bk][0:BL, :],
                            g2_sb[:, bk * NT:(bk + 1) * NT],
                        )
                        vector.drain()
                        vector.wait_ge(c1_sem, 2 * t + bk + 1)
                        if t >= 1:
                            vector.wait_ge(trC_sem, 2 * (t - 1) + bk + 1)
                        nc.vector.tensor_add(
                            c_sb[32:32 + BL, bk * NT:(bk + 1) * NT],
                            c1_sb[32:32 + BL, bk * NT:(bk + 1) * NT],
                            t1_sb[32:32 + BL, bk * NT:(bk + 1) * NT],
                        ).then_inc(csum_sem, 1)
                    for bk in range(2):
                        if t < t_steps - 1:
                            vector.wait_ge(trO_sem, 2 * t + bk + 1)
                            vector.wait_ge(tanh_sem, 2 * t + bk + 1)
                            if t >= 1:
                                vector.wait_ge(zmm_sem, 2 * (t - 1) + 2)
                            nc.vector.tensor_mul(
                                hT_sb[:, (t % 2) * (NK_H * BL) + bk * 32:
                                      (t % 2) * (NK_H * BL) + (bk + 1) * 32],
                                tr_o[:, bk * 32:(bk + 1) * 32],
                                tcT_sb[:, bk * 32:(bk + 1) * 32],
                            ).then_inc(hT_sem, 1)
                        else:
                            vector.wait_ge(tanh_sem, 2 * t + bk + 1)
                            nc.vector.tensor_mul(
                                hlast_sb[0:BL, bk * NT:(bk + 1) * NT],
                                s_sb[bk][64:64 + BL, :],
                                tc_sb[64:64 + BL, bk * NT:(bk + 1) * NT],
                            ).then_inc(hlast_sem, 1)

            @block.gpsimd
            def _(gpsimd):
                nc.gpsimd.memset(c_sb[32:32 + BL, :], 0.0)
                gpsimd.drain()
                for t in range(t_steps):
                    for bk in range(2):
                        # c1 = f * c, concurrent with DVE's g2/t1
                        gpsimd.wait_ge(sig_sem, 2 * t + bk + 1)
                        if t >= 1:
                            gpsimd.wait_ge(csum_sem, 2 * (t - 1) + bk + 1)
                        nc.gpsimd.tensor_mul(
                            c1_sb[32:32 + BL, bk * NT:(bk + 1) * NT],
                            s_sb[bk][32:32 + BL, :],
                            c_sb[32:32 + BL, bk * NT:(bk + 1) * NT],
                        ).then_inc(c1_sem, 1)

    return nc


def _prep_inputs(x, Wx, Wh, b, t_pad):
    """Host-side layout prep (pure layout/dtype, no compute)."""
    bf16 = ml_dtypes.bfloat16
    t_steps = x.shape[1]
    Wfull = np.concatenate([Wx, Wh, b[None, :]], axis=0).astype(np.float32)
    # original gate column ranges: i 0:U, f U:2U, g 2U:3U, o 3U:4U
    cols = []
    for bank in range(2):
        u0, u1 = bank * NT, (bank + 1) * NT
        cols.append(np.arange(0 * U + u0, 0 * U + u1))       # i
        cols.append(np.arange(1 * U + u0, 1 * U + u1))       # f
        cols.append(np.arange(3 * U + u0, 3 * U + u1))       # o
        cols.append(np.arange(2 * U + u0, 2 * U + u1))       # g
    perm = np.concatenate(cols)
    Wp = Wfull[:, perm].copy()
    # pre-scale g-gate columns by 2 (tanh(x) = 2*sigmoid(2x)-1)
    for bank in range(2):
        g0 = bank * 4 * NT + 3 * NT
        Wp[:, g0:g0 + NT] *= 2.0
    Wp = np.ascontiguousarray(Wp).astype(bf16)

    # per-core x, transposed + padded: xq[kc, p, t, b]
    xqs = []
    for core in range(NCORES):
        xs = x[core * BL:(core + 1) * BL].astype(np.float32)   # [BL, T, D]
        if t_pad != t_steps:
            pad = np.zeros((BL, t_pad - t_steps, xs.shape[2]), np.float32)
            xs = np.concatenate([xs, pad], axis=1)
        xt = np.ascontiguousarray(np.transpose(xs, (2, 1, 0)))  # [D, Tp, BL]
        xt = xt.reshape(NK_X, 128, t_pad, BL)
        xqs.append(np.ascontiguousarray(xt).astype(bf16))
    return Wp, xqs


def make_in_maps(x, Wx, Wh, b):
    t_steps = x.shape[1]
    t_pad = ((t_steps + TPT - 1) // TPT) * TPT
    Wp, xqs = _prep_inputs(x, Wx, Wh, b, t_pad)
    cst = np.zeros((72, 528), dtype=ml_dtypes.bfloat16)
    for i_ in range(BL):
        cst[i_, i_] = 1.0            # inject identity
        cst[64 + i_, 8 + i_] = 1.0   # o-transpose identity (base 64)
    cst[8, 0:8] = 1.0                # inject bias row
    cstf = np.zeros((72, 8), dtype=np.float32)
    for i_ in range(BL):
        cstf[32 + i_, i_] = 1.0  # f32 identity for the c transpose (base 32)
    return [{"wq": Wp, "xq": xqs[i], "cst": cst, "cstf": cstf}
            for i in range(NCORES)]


def kernel(x, Wx, Wh, b):
    x = np.asarray(x, dtype=np.float32)
    Wx = np.asarray(Wx, dtype=np.float32)
    Wh = np.asarray(Wh, dtype=np.float32)
    b = np.asarray(b, dtype=np.float32)
    t_steps = x.shape[1]

    nc = _build_nc(t_steps)

    from concourse.bass_utils import run_bass_kernel_spmd
    core_ids = list(range(NCORES))
    in_maps = make_in_maps(x, Wx, Wh, b)
    res = run_bass_kernel_spmd(nc, in_maps, core_ids,
                               trace=bool(globals().get("TRACE", False)))
    globals()["LAST_EXEC_NS"] = res.exec_time_ns

    h_parts = [res.results[i]["out"].astype(np.float32) for i in core_ids]
    return np.concatenate(h_parts, axis=0)


# revision 8
# speedup vs baseline: 1.3121x; 1.3121x over previous
"""Trainium2 Bass kernel for nn_BasicLSTM (B=64, T=512, D=512, U=1024).

Strategy: data-parallel over batch across 8 NeuronCores (8 sequences per
core, recurrence fully local per core — no cross-core communication).

Per-core step t computes z = [x_t, h, 1] @ W  as 13 K-chunks x 8 N-tiles of
512 columns.  N-tiles are spread over the PE array's four 32-row column
groups via tile_position, so four matmuls stream concurrently; the 8-row
(batch) outputs land at PSUM partition offsets {0,32,64,96}.  Weight columns
are host-permuted to [i|f|o|g] per 512-unit bank with the g-gate columns
pre-scaled by 2 so one sigmoid pass per bank covers every gate
(tanh(x) = 2*sigmoid(2x) - 1).  Weights, x (transposed host-side) and all
state live in SBUF for the whole kernel: the only DMAs are the initial
loads and the final store.  Matmul operands are bf16 (fp32 PSUM
accumulation); the cell state c stays fp32.
"""

import numpy as np
import ml_dtypes

B, T, D, U = 64, 512, 512, 1024
NCORES = 8
BL = B // NCORES          # 8 sequences per core
NK_X = D // 128           # 4 x K-chunks
NK_H = U // 128           # 8 h K-chunks
NT = 512                  # N-tile width (one PSUM bank)
GATE_OFF = (0, 32, 64, 96)  # PSUM partition offset per col-group (i,f,o,g)


def _build_nc(t_steps=T, lite=False):
    import concourse.bass as bass
    import concourse.mybir as mybir

    f32, bf16 = mybir.dt.float32, mybir.dt.bfloat16
    AF = mybir.ActivationFunctionType
    ALU = mybir.AluOpType

    nc = bass.Bass(num_devices=NCORES)
    if lite:
        # bench-only build: drop the big weight/x loads (timing is
        # data-independent); declare tiny dummies so transfer is cheap
        wq = nc.declare_dram_parameter("wq", [1, 64], bf16, isOutput=False)
        xq = nc.declare_dram_parameter("xq", [1, 64], bf16, isOutput=False)
    else:
        wq = nc.declare_dram_parameter("wq", [1537, 4096], bf16, isOutput=False)
        xq = nc.declare_dram_parameter("xq", [NK_X, 128, t_steps, BL], bf16, isOutput=False)
    ib_d = nc.declare_dram_parameter("ib", [BL + 1, BL], bf16, isOutput=False)
    op_d = nc.declare_dram_parameter("op_", [1, 128], bf16, isOutput=False)
    cz_d = nc.declare_dram_parameter("cz", [BL, U], f32, isOutput=False)
    out_d = nc.declare_dram_parameter("out", [BL, U], f32, isOutput=True)

    from contextlib import ExitStack
    ctx = ExitStack()
    sb = lambda shape, dt, name: ctx.enter_context(nc.sbuf_tensor(name, shape, dt))
    ps = lambda shape, dt, name: ctx.enter_context(nc.psum_tensor(name, shape, dt))
    sem = lambda name: ctx.enter_context(nc.semaphore(name))

    with ctx:
        w_sb = sb([128, 12 * 4096], bf16, "w_sb")
        bias_sb = sb([1, 4096], bf16, "bias_sb")
        x_sb = sb([128, NK_X * t_steps * BL], bf16, "x_sb")
        ones_sb = sb([1, BL], bf16, "ones_sb")
        ident = sb([BL, BL], bf16, "ident")
        s_sb = [sb([128, NT], bf16, f"s_sb{b}") for b in range(2)]
        g2_sb = [sb([BL, NT], bf16, f"g2_sb{b}") for b in range(2)]
        # operands of 2-input DVE ops must share a base partition; slice at
        # the offsets where the sigmoid output lives (f at 32, o at 64)
        t1_sb = [sb([32 + BL, NT], f32, f"t1_sb{b}") for b in range(2)]
        c1_sb = [sb([32 + BL, NT], f32, f"c1_sb{b}") for b in range(2)]
        tc_sb = [sb([64 + BL, NT], bf16, f"tc_sb{b}") for b in range(2)]
        h_sb = [sb([BL, NT], bf16, f"h_sb{b}") for b in range(2)]
        c_sb = sb([32 + BL, U], f32, "c_sb")
        hT_sb = sb([128, 2 * NK_H * BL], bf16, "hT_sb")  # double buffered h.T
        hlast_sb = sb([BL, U], f32, "hlast_sb")
        ones_pad = sb([1, 128], bf16, "ones_pad")

        zbuf = [ps([128, NT], f32, f"zbuf{i}") for i in range(4)]
        warm = ps([128, NT], f32, "warm")
        trbuf = [ps([128, 2 * NK_H * BL // 2], bf16, f"trbuf{i}") for i in range(2)]
        # trbuf: [128, 64] bf16; bank A transposes cols 0:32, bank B 32:64

        dma_sem = sem("dma_sem")
        mm_sem = sem("mm_sem")
        sig_sem = sem("sig_sem")
        csum_sem = sem("csum_sem")
        tanh_sem = sem("tanh_sem")
        h_sem = sem("h_sem")
        tr_sem = sem("tr_sem")
        cp_sem = sem("cp_sem")

        with nc.Block() as block:

            n_init_dmas = 4 if lite else (12 + 1 + NK_X + 4)

            @block.sync
            def _(sync):
                if not lite:
                    for kc in range(12):
                        sync.dma_start(
                            out=w_sb[:, kc * 4096:(kc + 1) * 4096],
                            in_=wq[kc * 128:(kc + 1) * 128, :],
                        ).then_inc(dma_sem, 16)
                    sync.dma_start(out=bias_sb[:, :], in_=wq[1536:1537, :]).then_inc(dma_sem, 16)
                    for kc in range(NK_X):
                        sync.dma_start(
                            out=x_sb[:, kc * (t_steps * BL):(kc + 1) * (t_steps * BL)],
                            in_=xq[kc],
                        ).then_inc(dma_sem, 16)
                sync.dma_start(out=ident[:, :], in_=ib_d[0:BL, :]).then_inc(dma_sem, 16)
                sync.dma_start(out=ones_sb[:, :], in_=ib_d[BL:BL + 1, :]).then_inc(dma_sem, 16)
                sync.dma_start(out=c_sb[32:32 + BL, :], in_=cz_d[:, :]).then_inc(dma_sem, 16)
                sync.dma_start(out=ones_pad[:, :], in_=op_d[:, :]).then_inc(dma_sem, 16)
                # final store
                sync.wait_ge(h_sem, 2 * t_steps)
                sync.dma_start(out=out_d[:, :], in_=hlast_sb[:, :]).then_inc(dma_sem, 16)

            @block.tensor
            def _(tensor):
                tensor.wait_ge(dma_sem, 16 * n_init_dmas)

                def xbias_mms(t):
                    zA = zbuf[(t % 2) * 2]
                    zB = zbuf[(t % 2) * 2 + 1]
                    for bk, z in ((0, zA), (1, zB)):
                        # bank opener: M=128 bias matmul via zero-padded ones
                        # row; writes bias into rows 0:8 of col-group 0 and
                        # ZEROS into all other partitions (clears the bank)
                        tensor.matmul(
                            z[:, :],
                            ones_pad[0:1, :],
                            bias_sb[0:1, (4 * bk) * NT:(4 * bk) * NT + NT],
                            start=True, stop=False,
                            skip_group_check=True,
                        )
                        for cg in range(1, 4):
                            ncol = (4 * bk + cg) * NT
                            tensor.matmul(
                                z[GATE_OFF[cg]:GATE_OFF[cg] + BL, :],
                                ones_sb[0:1, :],
                                bias_sb[0:1, ncol:ncol + NT],
                                start=False, stop=False,
                                tile_position=(0, GATE_OFF[cg]),
                                skip_group_check=True,
                            )
                        for kc in range(NK_X):
                            lhsT = x_sb[:, kc * (t_steps * BL) + t * BL:
                                        kc * (t_steps * BL) + (t + 1) * BL]
                            for cg in range(4):
                                ncol = (4 * bk + cg) * NT
                                last = (t == 0 and kc == NK_X - 1 and cg == 3)
                                ins = tensor.matmul(
                                    z[GATE_OFF[cg]:GATE_OFF[cg] + BL, :],
                                    lhsT,
                                    w_sb[:, kc * 4096 + ncol:kc * 4096 + ncol + NT],
                                    start=False, stop=last,
                                    tile_position=(0, GATE_OFF[cg]),
                                    skip_group_check=True,
                                )
                                if last:
                                    ins.then_inc(mm_sem, 1)

                def h_mms(t):
                    zA = zbuf[(t % 2) * 2]
                    zB = zbuf[(t % 2) * 2 + 1]
                    rd_buf = (t + 1) % 2
                    # unit chunks 0-3 come from copy_A of step t-1, 4-7 from
                    # copy_B -- start the first half as soon as copy_A lands.
                    # bank A's z must finish first (sigma_A first): bank-major
                    # within each copy half.
                    for half in range(2):
                        tensor.wait_ge(cp_sem, 2 * t - 1 + half)
                        for bk, z in ((0, zA), (1, zB)):
                            for j in range(half * 4, half * 4 + 4):
                                kc = NK_X + j
                                lhsT = hT_sb[:, rd_buf * (NK_H * BL) + j * BL:
                                             rd_buf * (NK_H * BL) + (j + 1) * BL]
                                for cg in range(4):
                                    ncol = (4 * bk + cg) * NT
                                    last = (half == 1 and j == 7 and cg == 3)
                                    ins = tensor.matmul(
                                        z[GATE_OFF[cg]:GATE_OFF[cg] + BL, :],
                                        lhsT,
                                        w_sb[:, kc * 4096 + ncol:kc * 4096 + ncol + NT],
                                        start=False, stop=last,
                                        tile_position=(0, GATE_OFF[cg]),
                                        skip_group_check=True,
                                    )
                                    if last:
                                        # pc-monotone completion: bank A's
                                        # earlier MMs are done by now
                                        ins.then_inc(mm_sem, 1)

                def transposes(t):
                    for bk in range(2):
                        tensor.wait_ge(h_sem, 2 * t + bk + 1)
                        for j in range(4):
                            ins = tensor.matmul(
                                trbuf[t % 2][:, (bk * 4 + j) * BL:(bk * 4 + j + 1) * BL],
                                h_sb[bk][0:BL, j * 128:(j + 1) * 128],
                                ident[:, :],
                                start=True, stop=True,
                                is_transpose=True,
                                skip_group_check=True,
                            )
                            if j == 3:
                                ins.then_inc(tr_sem, 1)

                def warm_mms(k):
                    # constant-input dummies keep the PE HAM un-throttled
                    # while waiting on the gate chain
                    for _ in range(k):
                        tensor.matmul(
                            warm[0:BL, :],
                            ones_sb[0:1, :],
                            bias_sb[0:1, 0:NT],
                            start=True, stop=True,
                            skip_group_check=True,
                        )

                # software pipeline: x/bias matmuls run two steps ahead
                xbias_mms(0)
                xbias_mms(1)
                for t in range(t_steps):
                    if t > 0:
                        h_mms(t)
                    if t + 2 < t_steps:
                        # openers overwrite banks last read by sigma(t)
                        tensor.wait_ge(sig_sem, 2 * t + 2)
                        xbias_mms(t + 2)
                    if t < t_steps - 1:
                        transposes(t)

            @block.scalar
            def _(scalar):
                for t in range(t_steps):
                    zA = zbuf[(t % 2) * 2]
                    zB = zbuf[(t % 2) * 2 + 1]
                    for bk, z in ((0, zA), (1, zB)):
                        scalar.wait_ge(mm_sem, 2 * t + bk + 1)
                        nc.scalar.activation(
                            s_sb[bk][:, :], z[:, :], mybir.ActivationFunctionType.Sigmoid,
                        ).then_inc(sig_sem, 1)
                    for bk in range(2):
                        scalar.wait_ge(csum_sem, 2 * t + bk + 1)
                        nc.scalar.activation(
                            tc_sb[bk][64:64 + BL, :], c_sb[32:32 + BL, bk * NT:(bk + 1) * NT],
                            mybir.ActivationFunctionType.Tanh,
                        ).then_inc(tanh_sem, 1)
                    if t < t_steps - 1:
                        for bk in range(2):
                            scalar.wait_ge(tr_sem, 2 * t + bk + 1)
                            nc.scalar.copy(
                                hT_sb[:, (t % 2) * (NK_H * BL) + bk * 4 * BL:
                                      (t % 2) * (NK_H * BL) + (bk + 1) * 4 * BL],
                                trbuf[t % 2][:, bk * 4 * BL:(bk + 1) * 4 * BL],
                            ).then_inc(cp_sem, 1)

            @block.vector
            def _(vector):
                ALU = mybir.AluOpType
                for t in range(t_steps):
                    # drain orders this step's reads after last step's writes
                    vector.drain()
                    for bk in range(2):
                        s = s_sb[bk]
                        vector.wait_ge(sig_sem, 2 * t + bk + 1)
                        nc.vector.tensor_scalar(
                            g2_sb[bk][:, :], s[96:96 + BL, :], 2.0, -1.0,
                            ALU.mult, ALU.add,
                        )
                        nc.vector.tensor_mul(
                            c1_sb[bk][32:32 + BL, :], s[32:32 + BL, :],
                            c_sb[32:32 + BL, bk * NT:(bk + 1) * NT],
                        )
                    vector.drain()
                    for bk in range(2):
                        nc.vector.tensor_mul(
                            t1_sb[bk][32:32 + BL, :], s_sb[bk][0:BL, :], g2_sb[bk][:, :])
                    vector.drain()
                    for bk in range(2):
                        nc.vector.tensor_add(
                            c_sb[32:32 + BL, bk * NT:(bk + 1) * NT],
                            c1_sb[bk][32:32 + BL, :], t1_sb[bk][32:32 + BL, :],
                        ).then_inc(csum_sem, 1)
                    for bk in range(2):
                        vector.wait_ge(tanh_sem, 2 * t + bk + 1)
                        if t < t_steps - 1:
                            nc.vector.tensor_mul(
                                h_sb[bk][:, :], s_sb[bk][64:64 + BL, :],
                                tc_sb[bk][64:64 + BL, :],
                            ).then_inc(h_sem, 1)
                        else:
                            nc.vector.tensor_mul(
                                hlast_sb[0:BL, bk * NT:(bk + 1) * NT],
                                s_sb[bk][64:64 + BL, :],
                                tc_sb[bk][64:64 + BL, :],
                            ).then_inc(h_sem, 1)

    return nc


def _prep_inputs(x, Wx, Wh, b):
    """Host-side layout prep (pure layout/dtype, no compute)."""
    bf16 = ml_dtypes.bfloat16
    t_steps = x.shape[1]
    # W = [Wx; Wh; b] rows, columns permuted to per-bank [i|f|o|g] blocks.
    Wfull = np.concatenate([Wx, Wh, b[None, :]], axis=0).astype(np.float32)
    # original gate column ranges: i 0:U, f U:2U, g 2U:3U, o 3U:4U
    cols = []
    for bank in range(2):
        u0, u1 = bank * NT, (bank + 1) * NT
        cols.append(np.arange(0 * U + u0, 0 * U + u1))       # i
        cols.append(np.arange(1 * U + u0, 1 * U + u1))       # f
        cols.append(np.arange(3 * U + u0, 3 * U + u1))       # o
        cols.append(np.arange(2 * U + u0, 2 * U + u1))       # g
    perm = np.concatenate(cols)
    Wp = Wfull[:, perm].copy()
    # pre-scale g-gate columns by 2 (tanh(x) = 2*sigmoid(2x)-1)
    for bank in range(2):
        g0 = bank * 4 * NT + 3 * NT
        Wp[:, g0:g0 + NT] *= 2.0
    Wp = np.ascontiguousarray(Wp).astype(bf16)

    # per-core x, transposed: xq[kc, p, t, b] = x[core*BL+b, t, kc*128+p]
    xqs = []
    for core in range(NCORES):
        xs = x[core * BL:(core + 1) * BL].astype(np.float32)      # [BL, T, D]
        xt = np.ascontiguousarray(np.transpose(xs, (2, 1, 0)))    # [D, T, BL]
        xt = xt.reshape(NK_X, 128, t_steps, BL)
        xqs.append(np.ascontiguousarray(xt).astype(bf16))
    return Wp, xqs


def make_in_maps(x, Wx, Wh, b):
    Wp, xqs = _prep_inputs(x, Wx, Wh, b)
    ib = np.zeros((BL + 1, BL), dtype=ml_dtypes.bfloat16)
    for i_ in range(BL):
        ib[i_, i_] = 1.0
    ib[BL, :] = 1.0
    cz = np.zeros((BL, U), dtype=np.float32)
    op = np.zeros((1, 128), dtype=ml_dtypes.bfloat16)
    op[0, :BL] = 1.0
    return [{"wq": Wp, "xq": xqs[i], "ib": ib, "cz": cz, "op_": op}
            for i in range(NCORES)]


def kernel(x, Wx, Wh, b):
    x = np.asarray(x, dtype=np.float32)
    Wx = np.asarray(Wx, dtype=np.float32)
    Wh = np.asarray(Wh, dtype=np.float32)
    b = np.asarray(b, dtype=np.float32)
    t_steps = x.shape[1]

    nc = _build_nc(t_steps)

    from concourse.bass_utils import run_bass_kernel_spmd
    core_ids = list(range(NCORES))
    in_maps = make_in_maps(x, Wx, Wh, b)
    res = run_bass_kernel_spmd(nc, in_maps, core_ids, trace=bool(globals().get("TRACE", False)))
    globals()["LAST_EXEC_NS"] = res.exec_time_ns

    # unshard: bank A = units 0:512, bank B = 512:1024 (identity unit order)
    h_parts = [res.results[i]["out"].astype(np.float32) for i in core_ids]
    return np.concatenate(h_parts, axis=0)

